# revision 10
# baseline (speedup 1.0000x reference)
"""Single-head attention (B=4, S=2048, D=E=1024) on 8 TRN2 NeuronCores.

Pair-transposed sharding: core c handles batch b = c//2 and KEY rows
h*1024:(h+1)*1024 with h = c%2, and computes scores/AV for BOTH query
halves of its pair against its local keys. Final per-query-half outputs
are produced by a pair ReduceScatter of the partial results.

Why this shape: the previous data-parallel-over-queries version gathered
K and V across the pair; the V gather landed mid-body right before the
AV matmuls and stalled the PE. Here the only collectives are
  - an early pair AllGather of Q^T (fp8, 1MB in / 2MB out), issued right
    after the Q projection and consumed ~55us of PE work later, and
  - a final pair ReduceScatter of the partial O^T (+denominator row),
    issued after the AV matmuls and consumed only by the output
    normalize, which in steady-state pipelining hides behind the next
    body's projections.
Neither sits before a dense PE phase, so the PE never waits on the wire.

On-chip layout (contraction dim on SBUF partitions everywhere):
  - host pre-transposes q (own query half) and k/v (own key half) to
    [D, 1024], bf16
  - projections produce Q^T [E, SQ] (fp8, shipped+kept), K^T [E, SKH]
    (fp8, local only), V [SKH, E] (bf16, local only) -- K/V never touch
    DRAM
  - scores S^T [sk_local, sq_pair=2048] in fp8 DoubleRow (256-wide
    contraction per matmul, ~2x); exp (no max subtraction; scores std
    ~1/3, |max| < ~2.5) -> E_s bf16
  - denominator partials via ones-vector matmul (stationary never
    changes -> ~zero LDWEIGHTS cost)
  - AV partial O^T [e, sq_pair] bf16; psums are copied straight into the
    ReduceScatter source laid out as [2, E+1, 1024]: block s = partial
    for query half s, row E = denominator partials. Each rank receives
    its own summed [E+1, 1024] block -- fully rank-symmetric, no
    partition-id addressing anywhere.
  - the ReduceScatter writes DIRECTLY into the ExternalOutput tensor
    (bf16, [E+1, SQ]); the final normalize (divide row-block by the
    denominator row) and transpose happen on the host in kernel().
    This keeps any post-collective work off the PE's in-order stream --
    the kernel's device program ends at the AV matmul drains.
Chains of 4 consecutive matmuls share each stationary tile in the
scores/AV phases (the moving dim is the 2048 pair queries = 4x512), vs
chains of 2 before: a stationary change costs ~+35ns on HW (walrus emits
LDWEIGHTS per matmul; reuse-adjacency is the only lever), so this also
trims ~8us of PE time.
"""

import sys

if "/opt/trn_rl_repo" not in sys.path:
    sys.path.insert(0, "/opt/trn_rl_repo")

import numpy as np
import ml_dtypes

P = 128
B, S, D, E = 4, 2048, 1024, 1024
SQ = 1024          # query rows per core (own half)
SQP = 2048         # pair query rows (own + partner)
SKH = 1024         # local key/value rows (own half)
SKTH = SKH // P    # 8
DO = D // P        # 8
EO = E // P        # 8
FD = 512           # matmul moving free dim
NQC = SQ // FD     # 2  (projection moving chunks)
NPC = SQP // FD    # 4  (scores/AV moving chunks over pair queries)
SCALE = 1.0 / np.sqrt(np.float32(E))

_NC_CACHE = {}


def _elide_redundant_ldweights(nc, mybir):
    """Post-scheduling pass: walk each basic block's PE instruction stream
    in final order; any matmul whose stationary AP equals the previous PE
    instruction's stationary AP keeps the already-loaded weights
    (ldweights=False)."""
    n_elided = 0
    for f in nc.m.functions:
        for bb in f.blocks:
            last_key = None
            for inst in bb.instructions:
                if isinstance(inst, mybir.InstLdweights):
                    last_key = repr(inst.ins[0])
                    continue
                if not isinstance(inst, mybir.InstMatmult):
                    continue
                if inst.is_transpose:
                    last_key = None
                    continue
                key = (repr(inst.ins[1]), inst.perf_mode)
                if last_key == key:
                    inst.ldweights = False
                    n_elided += 1
                else:
                    last_key = key
    return n_elided


def build_nc(loop_n=None, replicate_n=None, ldw_elide=False):
    """Build the per-core program (pair-transposed design).

    replicate_n: python-replicate the body N times in one NEFF (bench
    only; iterations overlap like steady-state pipelining, works with
    collectives)."""
    import concourse.bacc as bacc
    import concourse.mybir as mybir
    import concourse.tile as tile
    from concourse.bass import ts
    from contextlib import nullcontext

    bf16 = mybir.dt.bfloat16
    f32 = mybir.dt.float32
    fp8 = mybir.dt.float8e4
    DR = mybir.MatmulPerfMode.DoubleRow
    Exp = mybir.ActivationFunctionType.Exp
    mult = mybir.AluOpType.mult

    nc = bacc.Bacc("TRN2", target_bir_lowering=False, debug=False, num_devices=8)

    qT = nc.dram_tensor("qT", [D, SQ], bf16, kind="ExternalInput").ap()
    kT = nc.dram_tensor("kT", [D, SKH], bf16, kind="ExternalInput").ap()
    vT = nc.dram_tensor("vT", [D, SKH], bf16, kind="ExternalInput").ap()
    wq = nc.dram_tensor("wq", [D, E], bf16, kind="ExternalInput").ap()
    wk = nc.dram_tensor("wk", [D, E], bf16, kind="ExternalInput").ap()
    wv = nc.dram_tensor("wv", [D, E], bf16, kind="ExternalInput").ap()
    if replicate_n:
        # per-replica output slices so neuronx-cc can't dead-store-eliminate
        # the earlier replicas (bench-only shape)
        outT_full = nc.dram_tensor(
            "outT", [replicate_n, E + 1, SQ], bf16, kind="ExternalOutput").ap()
    else:
        outT = nc.dram_tensor("outT", [E + 1, SQ], bf16,
                              kind="ExternalOutput").ap()

    GROUPS = [[0, 1], [2, 3], [4, 5], [6, 7]]

    qT3 = qT.rearrange("(o p) s -> p o s", p=P)
    kT3 = kT.rearrange("(o p) s -> p o s", p=P)
    vT3 = vT.rearrange("(o p) s -> p o s", p=P)
    wq3 = wq.rearrange("(o p) e -> p o e", p=P)
    wk3 = wk.rearrange("(o p) e -> p o e", p=P)
    wv3 = wv.rearrange("(o p) e -> p o e", p=P)

    with tile.TileContext(nc) as tc:
        with tc.tile_pool(name="persist", bufs=1) as persist, \
             tc.tile_pool(name="epool", bufs=2) as epool, \
             tc.tile_pool(name="wpool", bufs=2) as wpool, \
             tc.tile_pool(name="stream", bufs=4) as stream, \
             tc.tile_pool(name="misc", bufs=1) as misc, \
             tc.tile_pool(name="ostage", bufs=2) as ostage, \
             tc.tile_pool(name="dram", bufs=2, space="DRAM") as dram, \
             tc.tile_pool(name="psum", bufs=6, space="PSUM") as psum, \
             (tc.For_i(0, loop_n, 1) if loop_n else nullcontext()):

            for _rep in range(replicate_n or 1):
                if replicate_n:
                    outT = outT_full[_rep]

                # ---- persistent on-chip tensors ---------------------------
                V_s = persist.tile([P, SKTH, E], bf16, tag="V")    # V[sk, e]
                # fp8 DoubleRow layout: e-tile et -> (group eg, half khi)
                # with et = 2*eg + khi; contraction pairs (partition, khi).
                KT_s = persist.tile([P, EO // 2, 2, SKH], fp8, tag="KT")
                QT_s = persist.tile([P, EO // 2, 2, SQP], fp8, tag="QT")
                # E_s double-buffered (epool): next body's scores/exp can
                # land while this body's AV matmuls still read E_s.
                E_s = epool.tile([P, SKTH, SQP], bf16, tag="EW")

                # [P, P] of ones: ones.T @ E gives the column sums
                # replicated on every output partition.
                ones = misc.tile([P, P], bf16, tag="ones")
                nc.any.memset(ones[:], 1.0)

                # DRAM bounce tiles (double-buffered via pool for
                # cross-body overlap of the collectives)
                kb_q = dram.tile([E, SQ], fp8, tag="kbq")
                gb_q = dram.tile([2, E, SQ], fp8, tag="gbq")
                kb_o = dram.tile([2, E + 1, SQ], bf16, tag="kbo")

                # ---- Q^T = (q @ Wq)^T, [e, sq_own], fp8, shipped ----------
                wq_s = wpool.tile([P, DO, E], bf16, tag="w")
                nc.sync.dma_start(wq_s[:], wq3)
                kb_q3 = kb_q.rearrange("(o p) s -> p o s", p=P)
                qcs = []
                for ci in range(NQC):
                    qc = stream.tile([P, DO, FD], bf16, tag="xtc",
                                     name=f"qc{ci}")
                    nc.sync.dma_start(qc[:], qT3[:, :, ts(ci, FD)])
                    qcs.append(qc)
                for et in range(EO):
                    pss = [psum.tile([P, FD], f32, tag="mm", name=f"ps{ci}")
                           for ci in range(NQC)]
                    for do in range(DO):
                        for ci in range(NQC):
                            nc.tensor.matmul(
                                pss[ci][:], wq_s[:, do, ts(et, P)],
                                qcs[ci][:, do, :],
                                start=(do == 0), stop=(do == DO - 1),
                            )
                    for ci in range(NQC):
                        qst = stream.tile([P, FD], fp8, tag="qst8")
                        nc.vector.tensor_copy(qst[:], pss[ci][:])
                        nc.sync.dma_start(kb_q3[:, et, ts(ci, FD)], qst[:])

                nc.gpsimd.collective_compute(
                    "AllGather",
                    mybir.AluOpType.bypass,
                    replica_groups=GROUPS,
                    ins=[kb_q.opt()],
                    outs=[gb_q.opt()],
                )

                # ---- K^T local half: [e, sk_local], fp8, on-chip ----------
                wk_s = wpool.tile([P, DO, E], bf16, tag="w")
                nc.sync.dma_start(wk_s[:], wk3)
                kcs = []
                for ci in range(NQC):
                    kc = stream.tile([P, DO, FD], bf16, tag="xtc",
                                     name=f"kc{ci}")
                    nc.sync.dma_start(kc[:], kT3[:, :, ts(ci, FD)])
                    kcs.append(kc)
                for et in range(EO):
                    pss = [psum.tile([P, FD], f32, tag="mm", name=f"ps{ci}")
                           for ci in range(NQC)]
                    for do in range(DO):
                        for ci in range(NQC):
                            nc.tensor.matmul(
                                pss[ci][:], wk_s[:, do, ts(et, P)],
                                kcs[ci][:, do, :],
                                start=(do == 0), stop=(do == DO - 1),
                            )
                    for ci in range(NQC):
                        nc.vector.tensor_copy(
                            KT_s[:, et // 2, et % 2, ts(ci, FD)], pss[ci][:])

                # ---- V local half: [sk_local, e], bf16, on-chip -----------
                wv_s = wpool.tile([P, DO, E], bf16, tag="w")
                nc.sync.dma_start(wv_s[:], wv3)
                for skt in range(SKTH):
                    vt = stream.tile([P, DO, P], bf16, tag="xtv")
                    nc.sync.dma_start(vt[:], vT3[:, :, ts(skt, P)])
                    pss = [psum.tile([P, FD], f32, tag="mm", name=f"ps{c}")
                           for c in range(E // FD)]
                    for do in range(DO):
                        for c in range(E // FD):
                            nc.tensor.matmul(
                                pss[c][:], vt[:, do, :], wv_s[:, do, ts(c, FD)],
                                start=(do == 0), stop=(do == DO - 1),
                            )
                    for c in range(E // FD):
                        nc.scalar.copy(V_s[:, skt, ts(c, FD)], pss[c][:])

                # ---- unpack gathered Q: slot s -> pair-query cols s*1024 --
                for s in range(2):
                    g_q3 = gb_q[s].rearrange("(o p) s -> p o s", p=P)
                    for half in range(2):
                        colslice = slice(s * SQ + half * FD,
                                         s * SQ + (half + 1) * FD)
                        nc.sync.dma_start(QT_s[:, :, :, colslice],
                                          g_q3[:, :, ts(half, FD)])

                # ---- E = exp(scale * S^T), S^T[sk_local, sq_pair] ---------
                # c-inner (chain 4) so consecutive matmuls share the
                # stationary lhsT
                for skt in range(SKTH):
                    pss = [psum.tile([P, FD], f32, tag="mm", name=f"ps{c}")
                           for c in range(NPC)]
                    for eg in range(EO // 2):
                        for c in range(NPC):
                            nc.tensor.matmul(
                                pss[c][:], KT_s[:, eg, :, ts(skt, P)],
                                QT_s[:, eg, :, ts(c, FD)],
                                start=(eg == 0), stop=(eg == EO // 2 - 1),
                                perf_mode=DR,
                            )
                    for c in range(NPC):
                        nc.scalar.activation(
                            E_s[:, skt, ts(c, FD)], pss[c][:], Exp,
                            scale=float(SCALE)
                        )

                # ---- denominator partials: den[sq] = sum_sk E[sk, sq] -----
                # ones.T @ E replicates the column sum on all 128
                # partitions; stationary (ones) never changes. Row E of
                # each kb_o block ships the partials through the
                # ReduceScatter.
                for c in range(NPC):
                    psd = psum.tile([P, FD], f32, tag="den", bufs=2)
                    for skt in range(SKTH):
                        nc.tensor.matmul(
                            psd[:], ones[:, :], E_s[:, skt, ts(c, FD)],
                            start=(skt == 0), stop=(skt == SKTH - 1),
                        )
                    dst = ostage.tile([1, FD], bf16, tag="dnst")
                    nc.vector.tensor_copy(dst[:], psd[0:1, :])
                    nc.sync.dma_start(
                        kb_o[c // 2, E, ts(c % 2, FD)], dst[:])

                # ---- partial O^T[e, sq_pair] = V^T E, ship ----------------
                for et in range(EO):
                    pss = [psum.tile([P, FD], f32, tag="mm", name=f"ps{c}")
                           for c in range(NPC)]
                    for skt in range(SKTH):
                        for c in range(NPC):
                            nc.tensor.matmul(
                                pss[c][:], V_s[:, skt, ts(et, P)],
                                E_s[:, skt, ts(c, FD)],
                                start=(skt == 0), stop=(skt == SKTH - 1),
                            )
                    for c in range(NPC):
                        ot = ostage.tile([P, FD], bf16, tag="ot")
                        nc.vector.tensor_copy(ot[:], pss[c][:])
                        nc.sync.dma_start(
                            kb_o[c // 2, ts(et, P), ts(c % 2, FD)], ot[:])

                # ReduceScatter: each rank receives its own summed
                # [E+1, SQ] block (O^T rows + den row). Collectives can't
                # write IO tensors, so land in an internal DRAM tile and
                # DMA-copy to the output -- still nothing on the PE's
                # in-order stream after the AV matmuls.
                gb_o = dram.tile([E + 1, SQ], bf16, tag="gbo")
                nc.gpsimd.collective_compute(
                    "ReduceScatter",
                    mybir.AluOpType.add,
                    replica_groups=GROUPS,
                    ins=[kb_o.opt()],
                    outs=[gb_o.opt()],
                )
                nc.sync.dma_start(outT, gb_o[:])

    if ldw_elide:
        n = _elide_redundant_ldweights(nc, mybir)
        print(f"ldweights elided: {n}")

    nc.compile()
    return nc


def get_nc():
    if "nc" not in _NC_CACHE:
        _NC_CACHE["nc"] = build_nc()
    return _NC_CACHE["nc"]


def make_in_maps(q, k, v, W_q, W_k, W_v):
    bf = ml_dtypes.bfloat16
    wq = np.ascontiguousarray(W_q.astype(bf))
    wk = np.ascontiguousarray(W_k.astype(bf))
    wv = np.ascontiguousarray(W_v.astype(bf))
    in_maps = []
    for c in range(8):
        b, h = c // 2, c % 2
        sl = slice(h * SKH, (h + 1) * SKH)
        qTc = np.ascontiguousarray(q[b, sl, :].astype(bf).T)
        kTc = np.ascontiguousarray(k[b, sl, :].astype(bf).T)
        vTc = np.ascontiguousarray(v[b, sl, :].astype(bf).T)
        in_maps.append({
            "qT": qTc, "kT": kTc, "vT": vTc,
            "wq": wq, "wk": wk, "wv": wv,
        })
    return in_maps


def kernel(q, k, v, W_q, W_k, W_v):
    from concourse import bass_utils

    q, k, v = np.asarray(q), np.asarray(k), np.asarray(v)
    W_q, W_k, W_v = np.asarray(W_q), np.asarray(W_k), np.asarray(W_v)
    nc = get_nc()
    in_maps = make_in_maps(q, k, v, W_q, W_k, W_v)
    res = bass_utils.run_bass_kernel_spmd(nc, in_maps, core_ids=list(range(8)))
    out = np.empty((B, S, E), dtype=np.float32)
    for c in range(8):
        b, h = c // 2, c % 2
        r = res.results[c]["outT"].astype(np.float32)
        out[b, h * SQ:(h + 1) * SQ, :] = (r[:E] / r[E:E + 1]).T
    return out
